# revision 6
# baseline (speedup 1.0000x reference)
"""Trainium2 Bass kernel for CharacterLevelSpectral.

Math: the reference embeds chars (x = char/255; emb = x*W + b broadcast over D),
FFTs along seq, zeroes mid frequencies (keeps lowest k=S/4 and highest k), IFFTs,
takes the real part.  The whole pipeline is linear along seq and the bias is
constant along seq (a constant's spectrum lives at f=0, which the low-pass
keeps), so

    out[b, s, d] = y[b, s] * W[d] + b[d],   y = lowpass(char/255)

and the FFT only needs to run on the (B, S) scalar signal, not (B, S, D).

y is computed per batch row with a factorized N1=128 x N2=64 Cooley-Tukey
FFT -> mask -> IFFT: small bf16 matmuls on the TensorEngine plus two
elementwise fp32 twiddle stages on the VectorEngine.  The frequency mask only
depends on f2 (k = 2048 = 16*128), so the DFT_64/mask/IDFT_64 stage collapses
into one precomputed 64x64 complex matrix G.

The memory-bound part is materializing the (2, 8192, 256) output per core.
It is stored as BF16 (8.4 MB/core instead of 16.8 MB fp32) and upcast to fp32
on the host - rel tolerance is 2e-2, bf16 rounding adds ~2e-3.  The broadcast
(out_chunk = y_col x W + b) runs on the TensorEngine as bf16 K=9 row-tiled
matmuls into 2-bank (1024-col) fp32 PSUM tiles; PSUM->SBUF copy-casts to bf16
staging alternate between ScalarE (ACT) and VectorE (DVE) with a 9:7 split
(DVE also runs the FFT twiddle stages); SBUF-only combine stages run on the
Pool engine (it has no PSUM port).  1MB staging tiles DMA out on the sync
queue with 8KB-contiguous per-partition descriptors; the first group-pair
streams out in 256KB chunks so the output DMA starts as early as possible.

Startup: three packed constant blocks load in PARALLEL on three HWDGE queues
(sync: chars+bf16 FFT consts; scalar: fp32 twiddles; gpsimd: W/b broadcast
block), so the ~2us per-DMA completion receipt is paid once, not 5x serial.
bb1's FFT stages are interleaved into bb0's broadcast pairs so the output
stream never pauses between batch rows.

Sharding: batch dim across 8 cores (2 rows per core), no cross-core traffic.
"""

import ml_dtypes
import numpy as np

import concourse.bass as bass
import concourse.mybir as mybir
import concourse.tile as tile
from concourse import bacc
from concourse.bass_utils import run_bass_kernel_spmd

B, S, D = 16, 8192, 256
NCORES = 8
BPC = B // NCORES  # batches per core
N1, N2 = 128, 64   # S = N1 * N2
KLP = S // 4       # low-pass cutoff
NG = 8             # chunks per broadcast group (K = NG + 1)

F32 = mybir.dt.float32
BF16 = mybir.dt.bfloat16
MULT = mybir.AluOpType.mult
ADD = mybir.AluOpType.add
SUB = mybir.AluOpType.subtract

# hb: single bf16 block: per-core chars (0..255 exact in bf16) + all bf16 DFT
# constants, ONE dma -> one ~2us completion receipt on the sync queue
HB_LAYOUT = {
    "m1re": (0, 128, 128, 128),
    "m1im": (0, 128, 256, 128),
    "m3re": (0, 128, 384, 128),
    "m3imn": (0, 128, 512, 128),
    "gre": (0, 64, 640, 64),
    "gim": (0, 64, 704, 64),
    "gimn": (0, 64, 768, 64),
}
HB_COLS = 832
# cb: single fp32 block on the scalar queue: [tw2p1|tw2p2] adjacent (tw2's
# fused multiply reads them as one [128,256] operand), [twtp1|twtp2] adjacent
# in rows 0:64 (tw1 reads [64,512])
CB_LAYOUT = {
    "tw2p1": (0, 128, 0, 128),
    "tw2p2": (0, 128, 128, 128),
    "twtp1": (0, 64, 256, 256),
    "twtp2": (0, 64, 512, 256),
}
CB_COLS = 768


def make_consts():
    """Input-independent DFT/twiddle constants, packed into two blocks."""
    n1 = np.arange(N1)
    n2 = np.arange(N2)
    C128 = np.cos(2 * np.pi * np.outer(n1, n1) / N1)
    S128 = np.sin(2 * np.pi * np.outer(n1, n1) / N1)
    kept = np.r_[0 : KLP // N1, N2 - KLP // N1 : N2]
    diff = n2[None, :] - n2[:, None]  # [n2, m2']: m2' - n2
    G = sum(np.exp(2j * np.pi * diff * f2 / N2) for f2 in kept)
    twtre = np.cos(2 * np.pi * np.outer(n2, n1) / S)    # [n2, f1]
    twtim = -np.sin(2 * np.pi * np.outer(n2, n1) / S)
    tw2re = np.cos(2 * np.pi * np.outer(n1, n2) / S)    # [f1, m2']
    tw2im = np.sin(2 * np.pi * np.outer(n1, n2) / S)
    c16 = {
        "m1re": C128 / 255.0,
        "m1im": -S128 / 255.0,
        "m3re": C128 / S,
        "m3imn": -S128 / S,
        "gre": G.real,
        "gim": G.imag,
        "gimn": -G.imag,
    }
    c32 = {
        "tw2p1": np.concatenate([tw2re, tw2im], axis=1),
        "tw2p2": np.concatenate([tw2im, tw2re], axis=1),
        "twtp1": np.concatenate([twtre, twtim], axis=1),
        "twtp2": np.concatenate([twtim, twtre], axis=1),
    }
    hb = np.zeros((N1, HB_COLS), dtype=np.float32)
    for name, (r0, rs, c0, cs) in HB_LAYOUT.items():
        hb[r0 : r0 + rs, c0 : c0 + cs] = c16[name]
    cb = np.zeros((N1, CB_COLS), dtype=np.float32)
    for name, (r0, rs, c0, cs) in CB_LAYOUT.items():
        cb[r0 : r0 + rs, c0 : c0 + cs] = c32[name]
    return hb.astype(ml_dtypes.bfloat16), cb


def build_program():
    """Build the per-core SPMD Bass program (identical on all cores)."""
    nc = bacc.Bacc("TRN2", target_bir_lowering=False, debug=False)

    hb_ext = nc.dram_tensor("hb", [N1, HB_COLS], BF16, kind="ExternalInput").ap()
    cb_ext = nc.dram_tensor("cb", [N1, CB_COLS], F32, kind="ExternalInput").ap()
    # 4 strip-replicas of [block-diag W | bias row], bf16
    wb4_ext = nc.dram_tensor("wb4", [105, NG * D], BF16, kind="ExternalInput").ap()
    # out[b, p, g, f] with s = 64*p + 8*g + f//256, d = f%256  — row-major
    # identical to (BPC, S, D); bf16, upcast on host
    out_ext = nc.dram_tensor("out", [BPC, N1, 8, 2048], BF16, kind="ExternalOutput").ap()

    with tile.TileContext(nc) as tc:
        with (
            tc.tile_pool(name="consts", bufs=1) as cpool,
            tc.tile_pool(name="work", bufs=2) as wpool,
            tc.tile_pool(name="stg", bufs=6) as spool,
            tc.tile_pool(name="ppfft", bufs=1, space="PSUM") as ppfft,
            tc.tile_pool(name="ppy", bufs=1, space="PSUM") as ppy,
            tc.tile_pool(name="ppb", bufs=3, space="PSUM") as ppb,
        ):
            # ---- three constant loads in PARALLEL on three queues ----
            hbt = cpool.tile([N1, HB_COLS], BF16)
            nc.sync.dma_start(out=hbt[:], in_=hb_ext)
            cbt = cpool.tile([N1, CB_COLS], F32)
            nc.scalar.dma_start(out=cbt[:], in_=cb_ext)
            wb4 = cpool.tile([105, NG * D], BF16)
            nc.gpsimd.dma_start(out=wb4[:], in_=wb4_ext)

            xall = hbt[:, 0 : 2 * N2]
            cs = {
                name: hbt[r0 : r0 + rs, c0 : c0 + cc]
                for name, (r0, rs, c0, cc) in HB_LAYOUT.items()
            }
            tw2pair = cbt[:, 0:256]            # [128, 256] = [tw2p1|tw2p2]
            twtpair = cbt[0:64, 256:768]       # [64, 512] = [twtp1|twtp2]

            # per-bb state cached between interleaved stages
            state = {}

            def fft_front(bb):
                """MM1 -> tw1 -> combines -> MM2 -> tw2 (up to uv2)."""
                xf = xall[:, bb * N2 : (bb + 1) * N2]
                # MM1: A'[n2, f1] = Xm.T @ M1 (re | im packed in free)
                apack = ppfft.tile([N2, 2 * N1], F32, tag="fftps")
                nc.tensor.matmul(apack[:, 0:N1], xf, cs["m1re"], start=True, stop=True)
                nc.tensor.matmul(apack[:, N1 : 2 * N1], xf, cs["m1im"], start=True, stop=True)
                # tw1 (DVE, psum read): uv = [are*twtre | aim*twtim | are*twtim | aim*twtre]
                uv = wpool.tile([N2, 4 * N1], F32, tag="uv")
                ap3 = (
                    apack[:]
                    .rearrange("p (o c) -> p o c", o=1)
                    .broadcast_to([N2, 2, 2 * N1])
                )
                nc.vector.tensor_tensor(
                    uv.rearrange("p (o c) -> p o c", o=2),
                    ap3,
                    twtpair.rearrange("p (o c) -> p o c", o=2),
                    MULT,
                )
                # combines (Pool, SBUF only)
                bre = wpool.tile([N2, N1], BF16, tag="bre")
                nc.gpsimd.tensor_tensor(bre[:], uv[:, 0:N1], uv[:, N1 : 2 * N1], SUB)
                bim = wpool.tile([N2, N1], BF16, tag="bim")
                nc.gpsimd.tensor_tensor(
                    bim[:], uv[:, 2 * N1 : 3 * N1], uv[:, 3 * N1 : 4 * N1], ADD
                )
                # MM2: Ck[f1, m2'] = B'.T @ G (re | im packed in free)
                ckpack = ppfft.tile([N1, 2 * N2], F32, tag="fftps")
                ckre, ckim = ckpack[:, 0:N2], ckpack[:, N2 : 2 * N2]
                nc.tensor.matmul(ckre, bre[:], cs["gre"], start=True, stop=False)
                nc.tensor.matmul(ckre, bim[:], cs["gimn"], start=False, stop=True)
                nc.tensor.matmul(ckim, bre[:], cs["gim"], start=True, stop=False)
                nc.tensor.matmul(ckim, bim[:], cs["gre"], start=False, stop=True)
                # tw2 (DVE, psum read):
                #   uv2 = [ckre*tw2re | ckim*tw2im | ckre*tw2im | ckim*tw2re]
                uv2 = wpool.tile([N1, 4 * N2], F32, tag="uv2")
                ck3 = (
                    ckpack[:]
                    .rearrange("p (o c) -> p o c", o=1)
                    .broadcast_to([N1, 2, 2 * N2])
                )
                nc.vector.tensor_tensor(
                    uv2.rearrange("p (o c) -> p o c", o=2),
                    ck3,
                    tw2pair.rearrange("p (o c) -> p o c", o=2),
                    MULT,
                )
                state[bb] = {"uv2": uv2}

            def fft_half(bb, half):
                """Combine + memsets + MM3 + ylhs copy for one 64-col half.

                ylhs[32g+c, p] = y[64p + 8(4*half+g) + c], ylhs[32g+8, :] = 1.
                """
                uv2 = state[bb]["uv2"]
                u2 = uv2[:, 0 : 2 * N2]
                v2 = uv2[:, 2 * N2 : 4 * N2]
                dmre = wpool.tile([N1, 128], BF16, tag=f"dmre{half}")
                dmim = wpool.tile([N1, 128], BF16, tag=f"dmim{half}")
                re3 = dmre.rearrange("p (g n) -> p g n", n=32)
                im3 = dmim.rearrange("p (g n) -> p g n", n=32)
                nc.gpsimd.memset(re3[:, :, NG:32], 0.0)
                nc.gpsimd.memset(im3[:, :, NG:32], 0.0)
                nc.gpsimd.memset(re3[0:1, :, NG : NG + 1], float(S))
                cols = slice(32 * half, 32 * half + 32)
                colsi = slice(N2 + 32 * half, N2 + 32 * half + 32)
                ua = u2[:, cols].rearrange("p (g c) -> p g c", c=NG)
                ub = u2[:, colsi].rearrange("p (g c) -> p g c", c=NG)
                nc.gpsimd.tensor_tensor(re3[:, :, 0:NG], ua, ub, SUB)
                va = v2[:, cols].rearrange("p (g c) -> p g c", c=NG)
                vb = v2[:, colsi].rearrange("p (g c) -> p g c", c=NG)
                nc.gpsimd.tensor_tensor(im3[:, :, 0:NG], va, vb, ADD)
                ylhs_ps = ppy.tile([N1, N1], F32, tag="ylhs_ps")
                nc.tensor.matmul(ylhs_ps[:], dmre[:], cs["m3re"], start=True, stop=False)
                nc.tensor.matmul(ylhs_ps[:], dmim[:], cs["m3imn"], start=False, stop=True)
                ylhs = wpool.tile([N1, N1], BF16, tag=f"ylhs{half}")
                nc.scalar.copy(ylhs[:], ylhs_ps[:])
                state[bb][f"ylhs{half}"] = ylhs

            # ACT/DVE copy rotation: 9 ACT, 7 DVE per 16 copies
            ACT_SLOTS = {0, 2, 4, 6, 8, 10, 12, 13, 14}
            copy_idx = [0]

            def copy_cast(dst, src):
                if (copy_idx[0] % 16) in ACT_SLOTS:
                    nc.scalar.copy(dst, src)
                else:
                    nc.vector.tensor_copy(dst, src)
                copy_idx[0] += 1

            def bcast_pair(bb, pair, stg, stg_off, flush):
                """Broadcast groups (2*pair, 2*pair+1): K=9 bf16 matmuls into
                2-bank psum tiles, copy-cast into bf16 staging at stg_off.

                flush: "early2" = stream each 1024-col h-chunk of both groups
                as soon as its two copies land (2 DMAs); "pair" = one 1MB DMA
                of this pair; "pair2" = one 2MB DMA of this + previous pair;
                "none" = caller flushes later.
                """
                gs = (2 * pair, 2 * pair + 1)
                ylhs = state[bb][f"ylhs{pair // 2}"]
                ps = {}
                for g in gs:
                    for h in range(2):
                        ps[g, h] = ppb.tile(
                            [N1, 1024], F32, tag="bcps", name=f"bcps{bb}_{g}_{h}"
                        )
                for h in range(2):
                    for q in (2 * h, 2 * h + 1):
                        for g in gs:
                            gp = 32 * (g % 4)  # partition strip
                            rows = slice(gp, gp + NG + 1)
                            nc.tensor.matmul(
                                ps[g, h][:, 512 * (q % 2) : 512 * (q % 2) + 512],
                                ylhs[rows, :],
                                wb4[rows, 512 * q : 512 * (q + 1)],
                                start=True,
                                stop=True,
                                tile_position=(gp, 0),
                            )
                    for g in gs:
                        gi = g - gs[0]
                        off = stg_off + 2048 * gi + 1024 * h
                        copy_cast(stg[:, off : off + 1024], ps[g, h][:])
                    if flush == "early2":
                        # [p, 2 groups, 1024] strided view of both h-chunks
                        chunk = stg[:, stg_off : stg_off + 4096].rearrange(
                            "p (g c) -> p g c", g=2
                        )[:, :, 1024 * h : 1024 * h + 1024]
                        nc.sync.dma_start(
                            out=out_ext[bb, :, gs[0] : gs[0] + 2,
                                        1024 * h : 1024 * h + 1024],
                            in_=chunk,
                        )
                if flush == "pair":
                    src = stg[:, stg_off : stg_off + 4096]
                    nc.sync.dma_start(
                        out=out_ext[bb, :, gs[0] : gs[0] + 2, :],
                        in_=src.rearrange("p (g c) -> p g c", g=2),
                    )
                elif flush == "pair2":
                    src = stg[:, stg_off - 4096 : stg_off + 4096]
                    nc.sync.dma_start(
                        out=out_ext[bb, :, gs[0] - 2 : gs[0] + 2, :],
                        in_=src.rearrange("p (g c) -> p g c", g=4),
                    )

            def stg_tile(name, ngroups):
                return spool.tile(
                    [N1, ngroups * NG * D], BF16, tag=f"stg{ngroups}", name=name
                )

            # ---- emission order: bb0 FFT front + half0 -> pair0 streams
            # ASAP; bb1's FFT stages interleave into bb0's later pairs so
            # PE/DVE/Pool stay fed and the DMA stream never pauses.  8 output
            # DMAs total (+3 input loads) stays within the DMA completion-
            # semaphore pool, avoiding ~2.5us recycle waits per issue. ----
            fft_front(0)
            fft_half(0, 0)
            s00 = stg_tile("s00", 2)
            bcast_pair(0, 0, s00, 0, "early2")
            fft_half(0, 1)
            s01 = stg_tile("s01", 2)
            bcast_pair(0, 1, s01, 0, "pair")
            fft_front(1)
            s02 = stg_tile("s02", 2)
            bcast_pair(0, 2, s02, 0, "pair")
            fft_half(1, 0)
            s03 = stg_tile("s03", 2)
            bcast_pair(0, 3, s03, 0, "pair")
            fft_half(1, 1)
            s10 = stg_tile("s10", 4)
            bcast_pair(1, 0, s10, 0, "none")
            bcast_pair(1, 1, s10, 4096, "pair2")
            s12 = stg_tile("s12", 2)
            bcast_pair(1, 2, s12, 0, "pair")
            s13 = stg_tile("s13", 2)
            bcast_pair(1, 3, s13, 0, "pair")

    nc.compile()
    return nc


_NC = None


def _get_nc():
    global _NC
    if _NC is None:
        _NC = build_program()
    return _NC


def make_in_maps(char_ids, W, b):
    char = np.asarray(char_ids).astype(np.float32)
    char = char.reshape(NCORES, BPC, N1, N2)
    wvec = np.asarray(W, dtype=np.float32)[:, 0]
    bvec = np.asarray(b, dtype=np.float32)
    wb9 = np.zeros((NG + 1, NG * D), dtype=np.float32)
    for c in range(NG):
        wb9[c, c * D : (c + 1) * D] = wvec
    wb9[NG] = np.tile(bvec, NG)
    wb4 = np.zeros((105, NG * D), dtype=np.float32)
    for g in range(4):
        wb4[32 * g : 32 * g + NG + 1] = wb9
    wb4 = wb4.astype(ml_dtypes.bfloat16)
    hbc, cb = make_consts()
    in_maps = []
    for i in range(NCORES):
        hb = np.array(hbc)
        for bb in range(BPC):
            hb[:, bb * N2 : (bb + 1) * N2] = char[i, bb].astype(ml_dtypes.bfloat16)
        in_maps.append({"hb": hb, "cb": cb, "wb4": wb4})
    return in_maps


def kernel(char_ids, W, b):
    nc = _get_nc()
    in_maps = make_in_maps(char_ids, W, b)
    res = run_bass_kernel_spmd(nc, in_maps, core_ids=list(range(NCORES)))
    parts = [r["out"].reshape(BPC, S, D) for r in res.results]
    return np.concatenate(parts, axis=0).astype(np.float32)
